# revision 2
# baseline (speedup 1.0000x reference)
"""Multi-head self-attention with RoPE on 8 Trainium2 NeuronCores. v2

Problem: B=2, S=2048, D=1024, H=16 heads, HD=64, causal, fp32.

Sharding: batch x head-group tensor parallel — core c owns batch c//4 and
heads 4*(c%4) .. 4*(c%4)+3 (two head-pairs). Host sums 4 partials per batch
and adds b_out (+ the V-bias term, which is exact post-softmax).

v2 design vs baseline:
- fp16 everywhere (2x DVE modes + better mantissa than bf16)
- 4 x 512-token chunks, software-pipelined emission: the projection of
  chunk ch+1 is emitted as PE filler between attention blocks of chunk ch
  (the per-block exp on ACT is slower than the block's PE work, so PE
  would otherwise idle ~250ns/block); out-projections of ch1/ch2 fill the
  last attention window, which has no projection left to overlap
- V computed token-major directly (qt tiles as stationary operand): no PE
  transposes, no V bias in-kernel (folded into host output: sum(P)=1)
- triangle causal masks on the idle GpSimd engine; raw copies on DVE so
  the ACT engine runs exp only
- 8-bank PSUM tag plan: sq(1) sk(1) [shared with rotate psums] +
  ps(2x2, scores/V/out-proj) + pa0(1) pa1(1) [PV accumulators]
- input DMAs split across both HWDGE queues, compute-first order
"""

import sys

if "/opt/trn_rl_repo" not in sys.path:
    sys.path.insert(0, "/opt/trn_rl_repo")

import numpy as np
import ml_dtypes

import concourse.bass as bass
import concourse.mybir as mybir
import concourse.tile as tile
from concourse import bacc
from concourse.bass_utils import run_bass_kernel_spmd

F32 = mybir.dt.float32
F16 = mybir.dt.float16
AF = mybir.ActivationFunctionType
ALU = mybir.AluOpType

B, S, D, H, HD = 2, 2048, 1024, 16, 64
NCORES = 8
GPB = NCORES // B              # head-groups per batch = 4
HPC = H // GPB                 # heads per core = 4 (2 pairs)
NP = HPC // 2                  # head pairs per core = 2
CW = HPC * HD                  # feature width per core = 256
CH = 512                       # token chunk
NCH = S // CH                  # 4
KT = D // 128                  # 8 contraction tiles
ROPE_BASE = 10000.0
SCALE = 1.0 / np.sqrt(HD)

_CACHED = {}


def build_nc(reps=1):
    nc = bacc.Bacc("TRN2", target_bir_lowering=False, debug=False,
                   num_devices=NCORES)

    qT = nc.dram_tensor("qT", [D, S], F16, kind="ExternalInput")
    wq = nc.dram_tensor("wq", [D, CW], F16, kind="ExternalInput")
    wk = nc.dram_tensor("wk", [D, CW], F16, kind="ExternalInput")
    wv = nc.dram_tensor("wv", [D, CW], F16, kind="ExternalInput")
    bq = nc.dram_tensor("bq", [128, NP], F32, kind="ExternalInput")
    bk = nc.dram_tensor("bk", [128, NP], F32, kind="ExternalInput")
    cosT = nc.dram_tensor("cosT", [64, S], F16, kind="ExternalInput")
    sinT = nc.dram_tensor("sinT", [64, S], F16, kind="ExternalInput")
    tri = nc.dram_tensor("tri", [128, 128], F16, kind="ExternalInput")
    rp = nc.dram_tensor("rp", [128, 128], F16, kind="ExternalInput")
    wout = nc.dram_tensor("wout", [CW, D], F16, kind="ExternalInput")
    outp = nc.dram_tensor("outp", [S, D], F16, kind="ExternalOutput")

    with tile.TileContext(nc) as tc:
        with (
            tc.tile_pool(name="const", bufs=1) as cpool,
            tc.tile_pool(name="persist", bufs=1) as ppool,
        ):
            # ---- constants resident in SBUF ----
            # SP queue, compute-critical order: wq (per-kt slices so the
            # first matmuls start after ~200ns of wire), cos, wk, wv, sin,
            # then the small tables and wout.
            wq_sb = cpool.tile([128, KT, CW], F16)
            wk_sb = cpool.tile([128, KT, CW], F16)
            wv_sb = cpool.tile([128, KT, CW], F16)
            cos_sb = cpool.tile([128, S], F16)
            sin_sb = cpool.tile([128, S], F16)
            wq_r = wq[:].rearrange("(a p) f -> p a f", p=128)
            wk_r = wk[:].rearrange("(a p) f -> p a f", p=128)
            wv_r = wv[:].rearrange("(a p) f -> p a f", p=128)
            nc.sync.dma_start(wq_sb[:, 0:4, :], wq_r[:, 0:4, :])
            nc.sync.dma_start(wk_sb[:, 0:4, :], wk_r[:, 0:4, :])
            nc.sync.dma_start(wq_sb[:, 4:8, :], wq_r[:, 4:8, :])
            nc.sync.dma_start(wk_sb[:, 4:8, :], wk_r[:, 4:8, :])
            nc.sync.dma_start(cos_sb[0:64, :], cosT[:])
            nc.vector.tensor_copy(cos_sb[64:128, :], cos_sb[0:64, :])
            nc.sync.dma_start(wv_sb[:], wv_r[:])
            nc.sync.dma_start(sin_sb[0:64, :], sinT[:])
            nc.vector.tensor_copy(sin_sb[64:128, :], sin_sb[0:64, :])
            tri_sb = cpool.tile([128, 128], F16)
            rp_sb = cpool.tile([128, 128], F16)
            bq_sb = cpool.tile([128, NP], F32)
            bk_sb = cpool.tile([128, NP], F32)
            wout_sb = cpool.tile([128, NP, D], F16)
            nc.sync.dma_start(rp_sb[:], rp[:])
            nc.sync.dma_start(tri_sb[:], tri[:])
            nc.sync.dma_start(bq_sb[:], bq[:])
            nc.sync.dma_start(bk_sb[:], bk[:])
            nc.sync.dma_start(wout_sb[:],
                              wout[:].rearrange("(g p) f -> p g f", p=128))

            # ---- persistent activations ----
            qf = ppool.tile([128, NP, S], F16)     # roped q, feature-major
            kf = ppool.tile([128, NP, S], F16)     # roped k, feature-major
            # token-major V per 128-token block: [V_hA(64)|ones(64)|V_hB(64)]
            vt = ppool.tile([128, NP, S // 128, 192], F16)
            at = ppool.tile([128, NP, S], F16)     # normalized attn^T

            nc.gpsimd.memset(vt[:, :, :, 64:128], 1.0)

            # PE warmup: dummy matmuls so the HAM clock-gate opens to
            # 2.4 GHz before the real work arrives (~3.4us of activity).
            # Outside the rep loop so benchmark reps don't pay for it.
            with tc.tile_pool(name="wu", bufs=1, space="PSUM") as wup:
                wu_a = cpool.tile([128, 512], F16)
                nc.vector.memset(wu_a[:, 0:256], 0.0)
                nc.vector.memset(wu_a[:, 256:512], 0.0)
                wu_ps = wup.tile([128, 512], F32)
                for i in range(10):
                    nc.tensor.matmul(wu_ps[:], wu_a[:, 0:128], wu_a[:],
                                     start=(i == 0), stop=(i == 9))

            _build_pipeline(nc, tc, locals(), reps)

    nc.compile()
    return nc


def _build_pipeline(nc, tc, env, reps):
    qT, outp = env["qT"], env["outp"]
    wq_sb, wk_sb, wv_sb = env["wq_sb"], env["wk_sb"], env["wv_sb"]
    wout_sb = env["wout_sb"]
    cos_sb, sin_sb = env["cos_sb"], env["sin_sb"]
    tri_sb, rp_sb = env["tri_sb"], env["rp_sb"]
    bq_sb, bk_sb = env["bq_sb"], env["bk_sb"]
    qf, kf, vt, at = env["qf"], env["kf"], env["vt"], env["at"]

    mm = nc.tensor.matmul

    with (
        tc.tile_pool(name="qt", bufs=2) as qtp,
        tc.tile_pool(name="raw", bufs=2) as rawp,
        tc.tile_pool(name="tmp", bufs=2) as tmpp,
        tc.tile_pool(name="exppool", bufs=6) as expp,
        tc.tile_pool(name="recip", bufs=2) as rcpp,
        tc.tile_pool(name="ostage", bufs=4) as ostp,
        tc.tile_pool(name="ps", bufs=1, space="PSUM") as psp,
    ):
        qT_r = qT[:].rearrange("(a p) f -> p a f", p=128)

        def qt_load(ch):
            eng = nc.scalar if ch == 0 else nc.sync
            chs = slice(CH * ch, CH * (ch + 1))
            qtc = qtp.tile([128, KT, CH], F16, tag="qtc", name="qtc")
            if ch == 0:
                # fine-grained so the first matmuls start early
                for i in range(4):
                    eng.dma_start(qtc[:, 2 * i:2 * i + 2, :],
                                  qT_r[:, 2 * i:2 * i + 2, chs])
            else:
                eng.dma_start(qtc[:, 0:4, :], qT_r[:, 0:4, chs])
                eng.dma_start(qtc[:, 4:8, :], qT_r[:, 4:8, chs])
            return [qtc[:, kt, :] for kt in range(KT)]

        def proj_units(ch, qts):
            """Emission closures for chunk ch's q/k/v projection + RoPE.
            Each unit is a contiguous burst of PE work usable as filler."""
            chs = slice(CH * ch, CH * (ch + 1))
            units = []
            state = {}

            def qk_group(p, w_sb, stag):
                pf = slice(128 * p, 128 * (p + 1))
                psx = psp.tile([128, CH], F32, tag=stag, name=stag)
                for kt in range(KT):
                    mm(psx[:], w_sb[:, kt, pf], qts[kt][:],
                       start=(kt == 0), stop=(kt == KT - 1))
                state[(stag, p)] = psx

            def rope(p, fx, bx, rtag, stag):
                # raw = X + b (fp16): feeds the rotate matmul, so the
                # rotated bias rides along for free
                psx = state[(stag, p)]
                raw = rawp.tile([128, CH], F16, tag=rtag, name=rtag)
                nc.vector.tensor_scalar_add(raw[:], psx[:], bx[:, p:p + 1])
                ps_r = psp.tile([128, CH], F32, tag=stag, name="rot")
                mm(ps_r[:], rp_sb[:], raw[:], start=True, stop=True)
                nc.vector.tensor_mul(fx[:, p, chs], raw[:], cos_sb[:, chs])
                tmp = tmpp.tile([128, CH], F16, tag="rtmp", name="rtmp")
                nc.vector.tensor_mul(tmp[:], ps_r[:], sin_sb[:, chs])
                nc.vector.tensor_add(fx[:, p, chs], fx[:, p, chs], tmp[:])

            def v_unit(tb):
                blk = 4 * ch + tb
                ps_v = psp.tile([128, CH], F32, tag=("sq", "sk")[tb % 2],
                                name="ps_v")
                pv = ps_v[:, 0:CW]
                for kt in range(KT):
                    mm(pv, qts[kt][:, 128 * tb:128 * (tb + 1)],
                       wv_sb[:, kt, :], start=(kt == 0), stop=(kt == KT - 1))
                dst = vt[:, :, blk, :].rearrange(
                    "q np (a b) -> q np a b", a=3)[:, :, ::2, :]
                src = pv.rearrange("q (np a b) -> q np a b", np=2, a=2)
                nc.vector.tensor_copy(dst, src)

            for p in range(NP):
                units.append(lambda p=p: qk_group(p, wq_sb, "sq"))
                units.append(lambda p=p: qk_group(p, wk_sb, "sk"))
                units.append(lambda p=p: rope(p, qf, bq_sb, "rq", "sq"))
                units.append(lambda p=p: rope(p, kf, bk_sb, "rk", "sk"))
            for tb in range(4):
                units.append(lambda tb=tb: v_unit(tb))
            return units

        def outproj_units(ch, act_copies=False, wide_psum=False):
            units = []

            def tt_unit(tt):
                trows = slice(128 * tt, 128 * (tt + 1))
                o_sb = ostp.tile([128, 1024], F16, tag="ost", name="ost")
                for nf in range(2):
                    fs = slice(512 * nf, 512 * (nf + 1))
                    if wide_psum:  # attention psums are free: 4-slot rotate
                        tag = (("sq", "sk"), ("pa0", "pa1"))[tt % 2][nf]
                    else:
                        tag = ("sq", "sk")[nf]
                    ps_o = psp.tile([128, CH], F32, tag=tag, name="ps_o")
                    for p in range(NP):
                        mm(ps_o[:], at[:, p, trows], wout_sb[:, p, fs],
                           start=(p == 0), stop=(p == NP - 1))
                    if act_copies and nf == 0:
                        nc.scalar.copy(o_sb[:, fs], ps_o[:])
                    else:
                        nc.vector.tensor_copy(o_sb[:, fs], ps_o[:])
                    if not wide_psum:
                        nc.sync.dma_start(outp[128 * tt:128 * (tt + 1), fs],
                                          o_sb[:, fs])
                if wide_psum:
                    # tail: one DMA per block, alternating hwdge queues so
                    # the final issues don't serialize on one sequencer
                    eng = (nc.sync, nc.scalar)[tt % 2]
                    eng.dma_start(outp[128 * tt:128 * (tt + 1), :], o_sb[:])

            for tt in range(4 * ch, 4 * ch + 4):
                units.append(lambda tt=tt: tt_unit(tt))
            return units

        def attn(ch, fillers):
            """Attention for chunk ch, popping one filler per block."""
            chs = slice(CH * ch, CH * (ch + 1))
            nblk = (4 * ch + 4) * 2
            fq = list(fillers)
            # spread fillers evenly across blocks
            sched = {}
            for i in range(len(fq)):
                sched.setdefault(int(i * nblk / max(len(fq), 1)), []).append(fq[i])
            bi = 0
            for p in range(NP):
                ph = [psp.tile([128, CH], F32, tag=f"pa{h}", name=f"pa{h}")
                      for h in range(2)]
                rmax = 4 * ch + 3
                for r in range(rmax + 1):
                    m = r - 4 * ch
                    ks_ = slice(128 * r, 128 * (r + 1))
                    ps_s = psp.tile([128, 1024], F32, tag="ps", name="ps_s",
                                    bufs=2)
                    for h in range(2):
                        p0 = 64 * h
                        mm(ps_s[:, 512 * h:512 * (h + 1)],
                           kf[p0:p0 + 64, p, ks_], qf[p0:p0 + 64, p, chs],
                           start=True, stop=True)
                    exp_sb = expp.tile([128, 1024], F16, tag="exp",
                                       name="exp")
                    if m <= 0:
                        nc.scalar.activation(exp_sb[:], ps_s[:], AF.Exp,
                                             scale=float(SCALE))
                    else:
                        src3 = ps_s[:].rearrange(
                            "p (a b) -> p a b", a=2)[:, :, 128 * m:512]
                        dst3 = exp_sb[:].rearrange(
                            "p (a b) -> p a b", a=2)[:, :, 128 * m:512]
                        nc.scalar.activation(dst3, src3, AF.Exp,
                                             scale=float(SCALE))
                    if m >= 0:
                        for h in range(2):
                            so = 512 * h + 128 * m
                            nc.vector.tensor_mul(exp_sb[:, so:so + 128],
                                                 exp_sb[:, so:so + 128],
                                                 tri_sb[:])
                    mm_ = max(m, 0)
                    for h in range(2):
                        # hA: [V|ones] -> rows 0-63 attn, 64-127 sums
                        # hB: [ones|V] -> rows 0-63 sums, 64-127 attn
                        mm(ph[h][:, 128 * mm_:CH],
                           vt[:, p, r, 64 * h:64 * h + 128],
                           exp_sb[:, 512 * h + 128 * mm_:512 * (h + 1)],
                           start=(r == 0), stop=(r == rmax))
                    for f in sched.get(bi, []):
                        f()
                    bi += 1
                # normalize in 256-col halves so the first out-proj blocks
                # of this chunk unblock sooner
                rc = rcpp.tile([128, CH], F32, tag="rc", name="rc")
                for cc in (slice(0, 256), slice(256, CH)):
                    nc.vector.reciprocal(rc[0:64, cc], ph[0][64:128, cc])
                    nc.vector.reciprocal(rc[64:128, cc], ph[1][0:64, cc])
                    nc.vector.tensor_mul(at[0:64, p, chs][:, cc],
                                         ph[0][0:64, cc], rc[0:64, cc])
                    nc.vector.tensor_mul(at[64:128, p, chs][:, cc],
                                         ph[1][64:128, cc], rc[64:128, cc])

        # ---- software-pipelined emission ----
        nxt = None
        for rep in range(reps):
            if nxt is None:
                qts = qt_load(0)
                for u in proj_units(0, qts):
                    u()
            else:
                for u in nxt:   # leftovers not emitted as attn3 fillers
                    u()
            qts = qt_load(1)
            attn(0, proj_units(1, qts))
            for u in outproj_units(0, act_copies=True):
                u()
            qts = qt_load(2)
            attn(1, proj_units(2, qts))
            qts = qt_load(3)
            attn(2, proj_units(3, qts))
            if rep + 1 < reps:
                # next rep's first projection fills this rep's big
                # attention window (and the tail outproj gaps)
                qts0 = qt_load(0)
                nxt = proj_units(0, qts0)
            else:
                nxt = []
            attn(3, outproj_units(1) + outproj_units(2) + nxt)
            nxt = []
            for u in outproj_units(3, act_copies=True, wide_psum=True):
                u()


def _host_prep(query, W_qkv, b_qkv, W_out, b_out):
    """Build per-core input maps. Core c: batch c//GPB, head-group c%GPB."""
    query = np.asarray(query, dtype=np.float32)
    qTb = [np.ascontiguousarray(query[b].T).astype(np.float16)
           for b in range(B)]

    inv_freq = 1.0 / (ROPE_BASE ** (np.arange(0, HD, 2, dtype=np.float32) / HD))
    freqs = np.arange(S, dtype=np.float32)[:, None] * inv_freq[None, :]
    emb = np.concatenate([freqs, freqs], axis=-1)          # (S, 64)
    cos = np.cos(emb).astype(np.float32).T                  # (64, S)
    sin = np.sin(emb).astype(np.float32).T
    sinp = sin.copy()
    sinp[0:32] = -sin[0:32]                                 # sign-folded
    cos128 = np.ascontiguousarray(cos).astype(np.float16)
    sin128 = np.ascontiguousarray(sinp).astype(np.float16)

    tri = np.ascontiguousarray(
        (np.arange(128)[None, :] >= np.arange(128)[:, None])
        .astype(np.float16))
    # rotate-half permutation: rot[m] = x[swap(m)] -> rp[k, m] = 1 iff
    # k == swap(m); swap exchanges 32-halves within each 64-block
    rp = np.zeros((128, 128), dtype=np.float16)
    for h in range(2):
        for i in range(64):
            rp[64 * h + (i + 32) % 64, 64 * h + i] = 1.0

    W_qkv = np.asarray(W_qkv, dtype=np.float32)
    b_qkv = np.asarray(b_qkv, dtype=np.float32)
    W_out = np.asarray(W_out, dtype=np.float32)

    def shift_bias(bb):
        out = bb.copy()
        for h in range(2):
            pq = 64 * h
            out[pq:pq + 32] = bb[pq + 32:pq + 64]
            out[pq + 32:pq + 64] = bb[pq:pq + 32]
        return out

    in_maps = []
    for c in range(NCORES):
        b = c // GPB
        g = c % GPB
        cols = slice(CW * g, CW * (g + 1))
        bqc = np.ascontiguousarray(b_qkv[0:D][cols].reshape(NP, 128).T)
        bkc = np.ascontiguousarray(b_qkv[D:2 * D][cols].reshape(NP, 128).T)
        in_maps.append({
            "qT": qTb[b],
            "wq": np.ascontiguousarray(W_qkv[:, 0:D][:, cols]).astype(np.float16),
            "wk": np.ascontiguousarray(W_qkv[:, D:2 * D][:, cols]).astype(np.float16),
            "wv": np.ascontiguousarray(W_qkv[:, 2 * D:3 * D][:, cols]).astype(np.float16),
            "bq": bqc,
            "bk": bkc,
            "cosT": cos128,
            "sinT": sin128,
            "tri": tri,
            "rp": rp,
            "wout": np.ascontiguousarray(W_out[CW * g:CW * (g + 1), :]).astype(np.float16),
        })
    return in_maps


def kernel(query, W_qkv, b_qkv, W_out, b_out):
    if "nc" not in _CACHED:
        _CACHED["nc"] = build_nc()
    nc = _CACHED["nc"]
    in_maps = _host_prep(query, W_qkv, b_qkv, W_out, b_out)
    res = run_bass_kernel_spmd(nc, in_maps, core_ids=list(range(NCORES)))
    acc = np.zeros((B, S, D), dtype=np.float64)
    for c, r in enumerate(res.results):
        acc[c // GPB] += np.asarray(r["outp"], dtype=np.float64)
    # V-bias is exact post-softmax (rows of P sum to 1): out += b_v @ W_out
    b_qkv = np.asarray(b_qkv, dtype=np.float64)
    bv_term = b_qkv[2 * D:3 * D] @ np.asarray(W_out, dtype=np.float64)
    acc += (bv_term + np.asarray(b_out, dtype=np.float64))[None, None, :]
    return acc.astype(np.float32)


# revision 3
# speedup vs baseline: 1.1317x; 1.1317x over previous
"""Multi-head self-attention with RoPE on 8 Trainium2 NeuronCores. v2

Problem: B=2, S=2048, D=1024, H=16 heads, HD=64, causal, fp32.

Sharding: batch x head-group tensor parallel — core c owns batch c//4 and
heads 4*(c%4) .. 4*(c%4)+3 (two head-pairs). Host sums 4 partials per batch
and adds b_out (+ the V-bias term, which is exact post-softmax).

v2 design vs baseline:
- fp16 everywhere (2x DVE modes + better mantissa than bf16)
- 4 x 512-token chunks, software-pipelined emission: the projection of
  chunk ch+1 is emitted as PE filler between attention blocks of chunk ch
  (the per-block exp on ACT is slower than the block's PE work, so PE
  would otherwise idle ~250ns/block); out-projections of ch1/ch2 fill the
  last attention window, which has no projection left to overlap
- V computed token-major directly (qt tiles as stationary operand): no PE
  transposes, no V bias in-kernel (folded into host output: sum(P)=1)
- triangle causal masks on the idle GpSimd engine; raw copies on DVE so
  the ACT engine runs exp only
- 8-bank PSUM tag plan: sq(1) sk(1) [shared with rotate psums] +
  ps(2x2, scores/V/out-proj) + pa0(1) pa1(1) [PV accumulators]
- input DMAs split across both HWDGE queues, compute-first order
"""

import sys

if "/opt/trn_rl_repo" not in sys.path:
    sys.path.insert(0, "/opt/trn_rl_repo")

import numpy as np
import ml_dtypes

import concourse.bass as bass
import concourse.mybir as mybir
import concourse.tile as tile
from concourse import bacc
from concourse.bass_utils import run_bass_kernel_spmd

F32 = mybir.dt.float32
F16 = mybir.dt.float16
AF = mybir.ActivationFunctionType
ALU = mybir.AluOpType

B, S, D, H, HD = 2, 2048, 1024, 16, 64
NCORES = 8
GPB = NCORES // B              # head-groups per batch = 4
HPC = H // GPB                 # heads per core = 4 (2 pairs)
NP = HPC // 2                  # head pairs per core = 2
CW = HPC * HD                  # feature width per core = 256
CH = 512                       # token chunk
NCH = S // CH                  # 4
KT = D // 128                  # 8 contraction tiles
ROPE_BASE = 10000.0
SCALE = 1.0 / np.sqrt(HD)

_CACHED = {}


def build_nc(reps=1):
    nc = bacc.Bacc("TRN2", target_bir_lowering=False, debug=False,
                   num_devices=NCORES)

    qT = nc.dram_tensor("qT", [D, S], F16, kind="ExternalInput")
    wq = nc.dram_tensor("wq", [D, CW], F16, kind="ExternalInput")
    wk = nc.dram_tensor("wk", [D, CW], F16, kind="ExternalInput")
    wv = nc.dram_tensor("wv", [D, CW], F16, kind="ExternalInput")
    bq = nc.dram_tensor("bq", [128, NP], F32, kind="ExternalInput")
    bk = nc.dram_tensor("bk", [128, NP], F32, kind="ExternalInput")
    cosT = nc.dram_tensor("cosT", [64, S], F16, kind="ExternalInput")
    sinT = nc.dram_tensor("sinT", [64, S], F16, kind="ExternalInput")
    tri = nc.dram_tensor("tri", [128, 128], F16, kind="ExternalInput")
    rp = nc.dram_tensor("rp", [128, 128], F16, kind="ExternalInput")
    wout = nc.dram_tensor("wout", [CW, D], F16, kind="ExternalInput")
    outp = nc.dram_tensor("outp", [S, D], F16, kind="ExternalOutput")

    with tile.TileContext(nc) as tc:
        with (
            tc.tile_pool(name="const", bufs=1) as cpool,
            tc.tile_pool(name="persist", bufs=1) as ppool,
        ):
            # ---- constants resident in SBUF ----
            # SP queue, compute-critical order: wq (per-kt slices so the
            # first matmuls start after ~200ns of wire), cos, wk, wv, sin,
            # then the small tables and wout.
            wq_sb = cpool.tile([128, KT, CW], F16)
            wk_sb = cpool.tile([128, KT, CW], F16)
            wv_sb = cpool.tile([128, KT, CW], F16)
            cos_sb = cpool.tile([128, S], F16)
            sin_sb = cpool.tile([128, S], F16)
            wq_r = wq[:].rearrange("(a p) f -> p a f", p=128)
            wk_r = wk[:].rearrange("(a p) f -> p a f", p=128)
            wv_r = wv[:].rearrange("(a p) f -> p a f", p=128)
            nc.sync.dma_start(wq_sb[:, 0:4, :], wq_r[:, 0:4, :])
            nc.sync.dma_start(wk_sb[:, 0:4, :], wk_r[:, 0:4, :])
            nc.sync.dma_start(wq_sb[:, 4:8, :], wq_r[:, 4:8, :])
            nc.sync.dma_start(wk_sb[:, 4:8, :], wk_r[:, 4:8, :])
            nc.sync.dma_start(cos_sb[0:64, :], cosT[:])
            nc.vector.tensor_copy(cos_sb[64:128, :], cos_sb[0:64, :])
            nc.sync.dma_start(wv_sb[:], wv_r[:])
            nc.sync.dma_start(sin_sb[0:64, :], sinT[:])
            nc.vector.tensor_copy(sin_sb[64:128, :], sin_sb[0:64, :])
            tri_sb = cpool.tile([128, 128], F16)
            rp_sb = cpool.tile([128, 128], F16)
            bq_sb = cpool.tile([128, NP], F32)
            bk_sb = cpool.tile([128, NP], F32)
            wout_sb = cpool.tile([128, NP, D], F16)
            nc.sync.dma_start(rp_sb[:], rp[:])
            nc.sync.dma_start(tri_sb[:], tri[:])
            nc.sync.dma_start(bq_sb[:], bq[:])
            nc.sync.dma_start(bk_sb[:], bk[:])
            nc.sync.dma_start(wout_sb[:],
                              wout[:].rearrange("(g p) f -> p g f", p=128))

            # ---- persistent activations ----
            qf = ppool.tile([128, NP, S], F16)     # roped q, feature-major
            kf = ppool.tile([128, NP, S], F16)     # roped k, feature-major
            # token-major V per 128-token block: [V_hA(64)|ones(64)|V_hB(64)]
            vt = ppool.tile([128, NP, S // 128, 192], F16)
            at = ppool.tile([128, NP, S], F16)     # normalized attn^T

            nc.gpsimd.memset(vt[:, :, :, 64:128], 1.0)

            # PE warmup: dummy matmuls so the HAM clock-gate opens to
            # 2.4 GHz before the real work arrives (~3.4us of activity).
            # Outside the rep loop so benchmark reps don't pay for it.
            with tc.tile_pool(name="wu", bufs=1, space="PSUM") as wup:
                wu_a = cpool.tile([128, 512], F16)
                nc.vector.memset(wu_a[:, 0:256], 0.0)
                nc.vector.memset(wu_a[:, 256:512], 0.0)
                wu_ps = wup.tile([128, 512], F32)
                for i in range(10):
                    nc.tensor.matmul(wu_ps[:], wu_a[:, 0:128], wu_a[:],
                                     start=(i == 0), stop=(i == 9))

            _build_pipeline(nc, tc, locals(), reps)

    nc.compile()
    return nc


def _build_pipeline(nc, tc, env, reps):
    qT, outp = env["qT"], env["outp"]
    wq_sb, wk_sb, wv_sb = env["wq_sb"], env["wk_sb"], env["wv_sb"]
    wout_sb = env["wout_sb"]
    cos_sb, sin_sb = env["cos_sb"], env["sin_sb"]
    tri_sb, rp_sb = env["tri_sb"], env["rp_sb"]
    bq_sb, bk_sb = env["bq_sb"], env["bk_sb"]
    qf, kf, vt, at = env["qf"], env["kf"], env["vt"], env["at"]

    mm = nc.tensor.matmul

    with (
        tc.tile_pool(name="qt", bufs=2) as qtp,
        tc.tile_pool(name="raw", bufs=2) as rawp,
        tc.tile_pool(name="tmp", bufs=2) as tmpp,
        tc.tile_pool(name="exppool", bufs=6) as expp,
        tc.tile_pool(name="recip", bufs=2) as rcpp,
        tc.tile_pool(name="ostage", bufs=4) as ostp,
        tc.tile_pool(name="ps", bufs=1, space="PSUM") as psp,
    ):
        qT_r = qT[:].rearrange("(a p) f -> p a f", p=128)

        def qt_load(ch):
            eng = nc.scalar if ch == 0 else nc.sync
            chs = slice(CH * ch, CH * (ch + 1))
            qtc = qtp.tile([128, KT, CH], F16, tag="qtc", name="qtc")
            if ch == 0:
                # fine-grained so the first matmuls start early
                for i in range(4):
                    eng.dma_start(qtc[:, 2 * i:2 * i + 2, :],
                                  qT_r[:, 2 * i:2 * i + 2, chs])
            else:
                eng.dma_start(qtc[:, 0:4, :], qT_r[:, 0:4, chs])
                eng.dma_start(qtc[:, 4:8, :], qT_r[:, 4:8, chs])
            return [qtc[:, kt, :] for kt in range(KT)]

        def proj_units(ch, qts):
            """Emission closures for chunk ch's q/k/v projection + RoPE.
            Each unit is a contiguous burst of PE work usable as filler."""
            chs = slice(CH * ch, CH * (ch + 1))
            units = []
            state = {}

            def qk_group(p, w_sb, stag):
                pf = slice(128 * p, 128 * (p + 1))
                psx = psp.tile([128, CH], F32, tag=stag, name=stag)
                for kt in range(KT):
                    mm(psx[:], w_sb[:, kt, pf], qts[kt][:],
                       start=(kt == 0), stop=(kt == KT - 1))
                state[(stag, p)] = psx

            def rope(p, fx, bx, rtag, stag):
                # raw = X + b (fp16): feeds the rotate matmul, so the
                # rotated bias rides along for free
                psx = state[(stag, p)]
                raw = rawp.tile([128, CH], F16, tag=rtag, name=rtag)
                nc.vector.tensor_scalar_add(raw[:], psx[:], bx[:, p:p + 1])
                ps_r = psp.tile([128, CH], F32, tag=stag, name="rot")
                mm(ps_r[:], rp_sb[:], raw[:], start=True, stop=True)
                nc.vector.tensor_mul(fx[:, p, chs], raw[:], cos_sb[:, chs])
                tmp = tmpp.tile([128, CH], F16, tag="rtmp", name="rtmp")
                nc.vector.tensor_mul(tmp[:], ps_r[:], sin_sb[:, chs])
                nc.vector.tensor_add(fx[:, p, chs], fx[:, p, chs], tmp[:])

            def v_unit(tb):
                blk = 4 * ch + tb
                ps_v = psp.tile([128, CH], F32, tag=("sq", "sk")[tb % 2],
                                name="ps_v")
                pv = ps_v[:, 0:CW]
                for kt in range(KT):
                    mm(pv, qts[kt][:, 128 * tb:128 * (tb + 1)],
                       wv_sb[:, kt, :], start=(kt == 0), stop=(kt == KT - 1))
                dst = vt[:, :, blk, :].rearrange(
                    "q np (a b) -> q np a b", a=3)[:, :, ::2, :]
                src = pv.rearrange("q (np a b) -> q np a b", np=2, a=2)
                nc.vector.tensor_copy(dst, src)

            for p in range(NP):
                units.append(lambda p=p: qk_group(p, wq_sb, "sq"))
                units.append(lambda p=p: qk_group(p, wk_sb, "sk"))
                units.append(lambda p=p: rope(p, qf, bq_sb, "rq", "sq"))
                units.append(lambda p=p: rope(p, kf, bk_sb, "rk", "sk"))
            for tb in range(4):
                units.append(lambda tb=tb: v_unit(tb))
            return units

        def outproj_units(ch, act_copies=False, wide_psum=False):
            units = []

            def tt_unit(tt):
                trows = slice(128 * tt, 128 * (tt + 1))
                o_sb = ostp.tile([128, 1024], F16, tag="ost", name="ost")
                for nf in range(2):
                    fs = slice(512 * nf, 512 * (nf + 1))
                    if wide_psum:  # attention psums are free: 4-slot rotate
                        tag = (("sq", "sk"), ("pa0", "pa1"))[tt % 2][nf]
                    else:
                        tag = ("sq", "sk")[nf]
                    ps_o = psp.tile([128, CH], F32, tag=tag, name="ps_o")
                    for p in range(NP):
                        mm(ps_o[:], at[:, p, trows], wout_sb[:, p, fs],
                           start=(p == 0), stop=(p == NP - 1))
                    if act_copies and nf == 0:
                        nc.scalar.copy(o_sb[:, fs], ps_o[:])
                    else:
                        nc.vector.tensor_copy(o_sb[:, fs], ps_o[:])
                    if not wide_psum:
                        nc.sync.dma_start(outp[128 * tt:128 * (tt + 1), fs],
                                          o_sb[:, fs])
                if wide_psum:
                    # tail: one DMA per block, alternating hwdge queues so
                    # the final issues don't serialize on one sequencer
                    eng = (nc.sync, nc.scalar)[tt % 2]
                    eng.dma_start(outp[128 * tt:128 * (tt + 1), :], o_sb[:])

            for tt in range(4 * ch, 4 * ch + 4):
                units.append(lambda tt=tt: tt_unit(tt))
            return units

        def attn(ch, fillers):
            """Attention for chunk ch, popping one filler per block."""
            chs = slice(CH * ch, CH * (ch + 1))
            nblk = (4 * ch + 4) * 2
            fq = list(fillers)
            # spread fillers evenly across blocks
            sched = {}
            for i in range(len(fq)):
                sched.setdefault(int(i * nblk / max(len(fq), 1)), []).append(fq[i])
            bi = 0
            for p in range(NP):
                ph = [psp.tile([128, CH], F32, tag=f"pa{h}", name=f"pa{h}")
                      for h in range(2)]
                rmax = 4 * ch + 3
                for r in range(rmax + 1):
                    m = r - 4 * ch
                    ks_ = slice(128 * r, 128 * (r + 1))
                    ps_s = psp.tile([128, 1024], F32, tag="ps", name="ps_s",
                                    bufs=2)
                    for h in range(2):
                        p0 = 64 * h
                        mm(ps_s[:, 512 * h:512 * (h + 1)],
                           kf[p0:p0 + 64, p, ks_], qf[p0:p0 + 64, p, chs],
                           start=True, stop=True)
                    exp_sb = expp.tile([128, 1024], F16, tag="exp",
                                       name="exp")
                    if m <= 0:
                        nc.scalar.activation(exp_sb[:], ps_s[:], AF.Exp,
                                             scale=float(SCALE))
                    else:
                        src3 = ps_s[:].rearrange(
                            "p (a b) -> p a b", a=2)[:, :, 128 * m:512]
                        dst3 = exp_sb[:].rearrange(
                            "p (a b) -> p a b", a=2)[:, :, 128 * m:512]
                        nc.scalar.activation(dst3, src3, AF.Exp,
                                             scale=float(SCALE))
                    if m >= 0:
                        for h in range(2):
                            so = 512 * h + 128 * m
                            nc.vector.tensor_mul(exp_sb[:, so:so + 128],
                                                 exp_sb[:, so:so + 128],
                                                 tri_sb[:])
                    mm_ = max(m, 0)
                    for h in range(2):
                        # hA: [V|ones] -> rows 0-63 attn, 64-127 sums
                        # hB: [ones|V] -> rows 0-63 sums, 64-127 attn
                        mm(ph[h][:, 128 * mm_:CH],
                           vt[:, p, r, 64 * h:64 * h + 128],
                           exp_sb[:, 512 * h + 128 * mm_:512 * (h + 1)],
                           start=(r == 0), stop=(r == rmax))
                    for f in sched.get(bi, []):
                        f()
                    bi += 1
                # normalize in 256-col halves so the first out-proj blocks
                # of this chunk unblock sooner
                rc = rcpp.tile([128, CH], F32, tag="rc", name="rc")
                for cc in (slice(0, 256), slice(256, CH)):
                    nc.vector.reciprocal(rc[0:64, cc], ph[0][64:128, cc])
                    nc.vector.reciprocal(rc[64:128, cc], ph[1][0:64, cc])
                    nc.vector.tensor_mul(at[0:64, p, chs][:, cc],
                                         ph[0][0:64, cc], rc[0:64, cc])
                    nc.vector.tensor_mul(at[64:128, p, chs][:, cc],
                                         ph[1][64:128, cc], rc[64:128, cc])

        # ---- software-pipelined emission ----
        for rep in range(reps):
            qts = qt_load(0)
            for u in proj_units(0, qts):
                u()
            qts = qt_load(1)
            attn(0, proj_units(1, qts))
            for u in outproj_units(0, act_copies=True):
                u()
            qts = qt_load(2)
            attn(1, proj_units(2, qts))
            qts = qt_load(3)
            attn(2, proj_units(3, qts))
            attn(3, outproj_units(1) + outproj_units(2))
            for u in outproj_units(3, act_copies=True, wide_psum=True):
                u()


def _host_prep(query, W_qkv, b_qkv, W_out, b_out):
    """Build per-core input maps. Core c: batch c//GPB, head-group c%GPB."""
    query = np.asarray(query, dtype=np.float32)
    qTb = [np.ascontiguousarray(query[b].T).astype(np.float16)
           for b in range(B)]

    inv_freq = 1.0 / (ROPE_BASE ** (np.arange(0, HD, 2, dtype=np.float32) / HD))
    freqs = np.arange(S, dtype=np.float32)[:, None] * inv_freq[None, :]
    emb = np.concatenate([freqs, freqs], axis=-1)          # (S, 64)
    cos = np.cos(emb).astype(np.float32).T                  # (64, S)
    sin = np.sin(emb).astype(np.float32).T
    sinp = sin.copy()
    sinp[0:32] = -sin[0:32]                                 # sign-folded
    cos128 = np.ascontiguousarray(cos).astype(np.float16)
    sin128 = np.ascontiguousarray(sinp).astype(np.float16)

    tri = np.ascontiguousarray(
        (np.arange(128)[None, :] >= np.arange(128)[:, None])
        .astype(np.float16))
    # rotate-half permutation: rot[m] = x[swap(m)] -> rp[k, m] = 1 iff
    # k == swap(m); swap exchanges 32-halves within each 64-block
    rp = np.zeros((128, 128), dtype=np.float16)
    for h in range(2):
        for i in range(64):
            rp[64 * h + (i + 32) % 64, 64 * h + i] = 1.0

    W_qkv = np.asarray(W_qkv, dtype=np.float32)
    b_qkv = np.asarray(b_qkv, dtype=np.float32)
    W_out = np.asarray(W_out, dtype=np.float32)

    def shift_bias(bb):
        out = bb.copy()
        for h in range(2):
            pq = 64 * h
            out[pq:pq + 32] = bb[pq + 32:pq + 64]
            out[pq + 32:pq + 64] = bb[pq:pq + 32]
        return out

    in_maps = []
    for c in range(NCORES):
        b = c // GPB
        g = c % GPB
        cols = slice(CW * g, CW * (g + 1))
        bqc = np.ascontiguousarray(b_qkv[0:D][cols].reshape(NP, 128).T)
        bkc = np.ascontiguousarray(b_qkv[D:2 * D][cols].reshape(NP, 128).T)
        in_maps.append({
            "qT": qTb[b],
            "wq": np.ascontiguousarray(W_qkv[:, 0:D][:, cols]).astype(np.float16),
            "wk": np.ascontiguousarray(W_qkv[:, D:2 * D][:, cols]).astype(np.float16),
            "wv": np.ascontiguousarray(W_qkv[:, 2 * D:3 * D][:, cols]).astype(np.float16),
            "bq": bqc,
            "bk": bkc,
            "cosT": cos128,
            "sinT": sin128,
            "tri": tri,
            "rp": rp,
            "wout": np.ascontiguousarray(W_out[CW * g:CW * (g + 1), :]).astype(np.float16),
        })
    return in_maps


def kernel(query, W_qkv, b_qkv, W_out, b_out):
    if "nc" not in _CACHED:
        _CACHED["nc"] = build_nc()
    nc = _CACHED["nc"]
    in_maps = _host_prep(query, W_qkv, b_qkv, W_out, b_out)
    res = run_bass_kernel_spmd(nc, in_maps, core_ids=list(range(NCORES)))
    acc = np.zeros((B, S, D), dtype=np.float64)
    for c, r in enumerate(res.results):
        acc[c // GPB] += np.asarray(r["outp"], dtype=np.float64)
    # V-bias is exact post-softmax (rows of P sum to 1): out += b_v @ W_out
    b_qkv = np.asarray(b_qkv, dtype=np.float64)
    bv_term = b_qkv[2 * D:3 * D] @ np.asarray(W_out, dtype=np.float64)
    acc += (bv_term + np.asarray(b_out, dtype=np.float64))[None, None, :]
    return acc.astype(np.float32)
